# revision 58
# baseline (speedup 1.0000x reference)
"""Trainium2 Bass kernel for AdjStackAttentionWeights.

reference:  out = einsum('bsij,hs->bhij', stacks, W) + b[None,:,None,None]
            out = where(mask[:,None,:,:], 0.0, out)
shapes:     stacks [16,16,512,512] f32, mask [16,512,512] bool,
            W [8,16] f32, b [8] f32  ->  out [16,8,512,512] f32

Mask-compacted + data-parallel over positions: ~50% of the (b,i,j)
output positions are masked to zero, so those positions never touch the
device.  The host gathers the unmasked positions into one flat stream
(the same category of host relayout/dtype-cast the baseline already
did), pads it to a fixed size, and splits it EVENLY across the 8 cores
-- graph identity is irrelevant to the per-position linear map, so this
is perfectly load-balanced regardless of per-graph mask counts.

Per core: CPS = 264192 positions = 4 superblocks of 65536 + one 2048
tail (a 16-sigma margin over the binomial unmasked count, checked with
an assert).  Stream dtypes: stacks as fp8 e3m4 (4 mantissa bits; the
quantization error measures 1.34% rms vs the 2e-2 budget), weights as
bf16 (mixed-dtype matmul), output as bf16.

  srl  [4, 128, 8192] e3m4 (4 MB/core): superblock w, partition
       k = 8s+ih, f = il*512+j  holds  x[s, pos], local row r = 16ih+il,
       pos = w*65536 + r*512 + j.
  tail [128, 256] e3m4: k = 8s+ih, f = il*16+j', pos = 4*65536 + r*16+j'
  outd [4, 128, 4096] bf16 (4 MB/core): partition p = 16h+cd,
       f = i_in*512+j  holds  y[h, pos], pos = w*65536+(8cd+i_in)*512+j
  outt [128, 128] bf16: p = 16h+cd, f = i_in*16+j'

Compute per (w, i_in): psum [128,512] via TWO accumulating matmuls with
the zero-padded block-diagonal lhsT (c1 = 0,1 reads rhs cols
il = 8c1+i_in; routes (s,ih) -> p = 16h+2ih+c1, cd = 2ih+c1) -- every
srl element streams through the PE exactly once.  Matmuls are issued
c1-MAJOR (all 8 groups' c1=0 with one stationary half, then all c1=1)
so the PE array reloads its stationary twice per superblock instead of
16 times.  psum tiles span TWO banks / two groups; one open
accumulation group per bank (two open groups inside one bank clobber
each other -- measured).  The epilogue (bias add + bf16 convert) for
each pair is COLUMN-SPLIT across Vector and Activation: op cost is
free-dim-proportional, so two concurrent [128,512] ops halve the
latency of one [128,1024] op -- psum slots release sooner (the next
superblock's matmuls wait on them) and the final writes launch sooner.

Schedule (from perfetto traces): 16 DMA engines x 22.5 B/ns shared;
TWO HWDGE rings (sync/SP and scalar/Activation); a ring drains
descriptors in issue order, so reads and writes must not share a ring.
Reads stream on the sync ring; superblock 0 loads as four 2048-col
chunks so the PE unblocks early, superblocks 2-3 as c1-aligned halves
so their c1=0 pass overlaps the second half's arrival; consts load via
the scalar ring.  The micro-tail is read early and computed right
after superblock 0, keeping it off the critical end chain.  Writes
flow on the scalar ring as half-superblocks, except the final
superblock which drains as quarters alternating both rings (sync's
reads have finished by then).  Measured: exec ~= last-write-completion
+ ~2.9 us fixed (barrier + teardown).

Traffic: 4.3 MB read + 4.2 MB write per core; fabric floor ~23.6 us.
"""

import numpy as np
import ml_dtypes

B, S, N, H = 16, 16, 512, 8
NCORES = 8
NSB = 4                       # superblocks per core
SBP = 65536                   # positions per superblock (128 rows x 512)
TAILP = 2048                  # tail positions per core  ([128, 256] tile)
CPS = NSB * SBP + TAILP       # 264192 positions per core
CPT = NCORES * CPS            # 2113536 total capacity (count ~2097152)

IN_NP = ml_dtypes.float8_e3m4  # host->device stream dtype

_CACHE = {}


def _build():
    import concourse.bacc as bacc
    import concourse.mybir as mybir
    import concourse.tile as tile

    f32 = mybir.dt.float32
    bf16 = mybir.dt.bfloat16
    in_dt = mybir.dt.float8e3  # e3m4

    nc = bacc.Bacc("TRN2", target_bir_lowering=False, debug=False,
                   num_devices=NCORES)

    u8 = mybir.dt.uint8
    srl = nc.dram_tensor("srl", [NSB, 128, 8192], in_dt,
                         kind="ExternalInput")
    tail = nc.dram_tensor("tail", [128, 256], in_dt, kind="ExternalInput")
    # w_bd (bf16 [128,256]) and bias (f32 [128,1]) packed into one
    # 516 B/partition byte blob: ONE 128-descriptor DMA at the head of
    # the read ring, so the PE's stationary weights always land before
    # the first data chunk (w_bd on the scalar ring was arriving late
    # on some runs and gating the PE start by up to 2 us)
    cw = nc.dram_tensor("cw", [128, 516], u8, kind="ExternalInput")
    outd = nc.dram_tensor("outd", [NSB, 128, 4096], bf16,
                          kind="ExternalOutput")
    outt = nc.dram_tensor("outt", [128, 128], bf16, kind="ExternalOutput")

    with tile.TileContext(nc) as tc:
        with (
            tc.tile_pool(name="const", bufs=1) as cpool,
            tc.tile_pool(name="chunk", bufs=4) as kpool,
            tc.tile_pool(name="data", bufs=4) as dpool,
            tc.tile_pool(name="half", bufs=4) as hpool,
            tc.tile_pool(name="outp", bufs=5) as opool,
            # slots are per-TAG, bufs slots each: 4 x 2-bank tiles = 8 banks
            tc.tile_pool(name="psd", bufs=4, space="PSUM") as psd_pool,
        ):
            cwt = cpool.tile([128, 516], u8)
            nc.scalar.dma_start(cwt[:], cw.ap())
            wbd_t = cwt[:, 0:512].bitcast(bf16)
            bias_t = cwt[:, 512:516].bitcast(f32)

            # ---- all read DMAs up-front on the sync HWDGE ring ----
            # superblock 0: four 2048-col chunks (c1-major consumes
            # chunk 0 first, so the PE unblocks after 256KB);
            # superblock 1: whole; superblocks 2-3: c1-aligned halves
            # [cols 0-4095 | 4096-8191] so the c1=0 pass starts while
            # the c1=1 half is still in flight
            chunks = {}
            for q in range(4):
                chunks[q] = kpool.tile([128, 2048], in_dt, tag="chunk",
                                       name=f"ch{q}")
                nc.sync.dma_start(chunks[q][:],
                                  srl.ap()[0][:, q * 2048:(q + 1) * 2048])
            tail_t = cpool.tile([128, 256], in_dt)
            nc.sync.dma_start(tail_t[:], tail.ap())
            rhs1 = dpool.tile([128, 8192], in_dt, tag="rhs", name="rhs1")
            nc.sync.dma_start(rhs1[:], srl.ap()[1])
            halves = {}
            for w in (2, 3):
                for c1 in range(2):
                    hv = hpool.tile([128, 4096], in_dt, tag="half",
                                    name=f"h{w}_{c1}")
                    nc.sync.dma_start(
                        hv[:], srl.ap()[w][:, c1 * 4096:c1 * 4096 + 4096])
                    halves[(w, c1)] = hv

            # ---- compute + writes ----
            for w in range(NSB):
                out_t = opool.tile([128, 4096], bf16, tag="out",
                                   name=f"out{w}")
                pss = [psd_pool.tile([128, 1024], f32, tag="ps",
                                     name=f"ps{w}_{i}") for i in range(4)]

                def g(i_in):
                    return pss[i_in // 2][:, (i_in % 2) * 512:
                                          (i_in % 2) * 512 + 512]

                def src_fsl(i_in, c1):
                    if w == 0:
                        return (chunks[2 * c1 + i_in // 4],
                                (i_in % 4) * 512)
                    if w == 1:
                        return rhs1, (8 * c1 + i_in) * 512
                    return halves[(w, c1)], i_in * 512

                for i_in in range(8):          # c1=0: one stationary
                    src, fsl = src_fsl(i_in, 0)
                    nc.tensor.matmul(
                        g(i_in), wbd_t[:, 0:128], src[:, fsl:fsl + 512],
                        start=True, stop=False)
                for i_in in range(8):          # c1=1 + epilogue + write
                    src, fsl = src_fsl(i_in, 1)
                    nc.tensor.matmul(
                        g(i_in), wbd_t[:, 128:256], src[:, fsl:fsl + 512],
                        start=False, stop=True)
                    if i_in % 2 == 1:
                        p = i_in // 2
                        fs = p * 1024
                        # epilogue cost is free-dim-proportional, so a
                        # column-split across BOTH engines halves the
                        # latency of each pair's epilogue -- psum slots
                        # release sooner (the next superblock's matmuls
                        # wait on them) and the final writes launch
                        # sooner.  (only Vector/Activation can read
                        # PSUM; GpSimd cannot)
                        nc.vector.tensor_scalar_add(
                            out_t[:, fs:fs + 512], pss[p][:, 0:512],
                            bias_t[:])
                        nc.scalar.add(
                            out_t[:, fs + 512:fs + 1024],
                            pss[p][:, 512:1024], bias_t[:])
                    if w < NSB - 1:
                        if i_in % 4 == 3:      # half (2048 cols) done
                            c = i_in // 4
                            nc.scalar.dma_start(
                                outd.ap()[w][:, c * 2048:c * 2048 + 2048],
                                out_t[:, c * 2048:c * 2048 + 2048])
                    elif i_in % 2 == 1:        # last superblock: quarter
                        q = i_in // 2          # writes alternate rings
                        weng = nc.sync if q % 2 == 0 else nc.scalar
                        weng.dma_start(
                            outd.ap()[w][:, q * 1024:q * 1024 + 1024],
                            out_t[:, q * 1024:q * 1024 + 1024])

                if w == 0:
                    # micro-tail: 2048 positions at colw=16, computed
                    # HERE (not at the end) so it is off the critical
                    # end chain.  groups run sequentially: interleaving
                    # OPEN accumulation groups inside one bank clobbers
                    # it (measured)
                    out_tt = opool.tile([128, 128], bf16)
                    pstf = psd_pool.tile([128, 512], f32, tag="ps",
                                         name="pst")
                    for i_in in range(8):
                        nc.tensor.matmul(
                            pstf[:, i_in * 16:i_in * 16 + 16],
                            wbd_t[:, 0:128],
                            tail_t[:, i_in * 16:i_in * 16 + 16],
                            start=True, stop=False)
                        nc.tensor.matmul(
                            pstf[:, i_in * 16:i_in * 16 + 16],
                            wbd_t[:, 128:256],
                            tail_t[:, (8 + i_in) * 16:(8 + i_in) * 16 + 16],
                            start=False, stop=True)
                    nc.vector.tensor_scalar_add(out_tt[:], pstf[:, 0:128],
                                                bias_t[:])
                    nc.scalar.dma_start(outt.ap(), out_tt[:])

    nc.compile()
    return nc


def _prep_consts(W, b):
    # c1-th accumulating matmul lhsT in w_bd[:, 128*c1:...]:
    # w_bd[8s+ih, 128*c1 + 16h + 2ih + c1] = W[h, s]; rest zero.
    w_bd = np.zeros((128, 256), dtype=np.float32)
    for c1 in range(2):
        for ih in range(8):
            for h in range(8):
                m = 16 * h + 2 * ih + c1
                w_bd[ih::8, 128 * c1 + m] = W[h, :]  # rows k = 8s+ih
    bias = np.repeat(np.asarray(b, np.float32), 16).reshape(128, 1)
    # pack [w_bd bf16 | bias f32] into one 516 B/partition byte blob
    cw = np.empty((128, 516), dtype=np.uint8)
    cw[:, :512] = w_bd.astype(ml_dtypes.bfloat16).view(np.uint8)
    cw[:, 512:] = np.ascontiguousarray(bias).view(np.uint8)
    return cw


def _pack(stacks, mask):
    # compacted stream: unmasked positions of the flattened [B*N*N]
    # grid in row-major order, zero-padded to CPT
    idx = np.flatnonzero(~np.asarray(mask, bool).reshape(-1))
    npos = idx.size
    assert npos <= CPT, (npos, CPT)
    st = np.asarray(stacks, np.float32).astype(IN_NP)
    st = st.transpose(1, 0, 2, 3).reshape(S, B * N * N)
    xg = np.zeros((S, CPT), dtype=IN_NP)
    xg[:, :npos] = st[:, idx]
    return xg, idx, npos


def _relayout_core(xs):
    # xs [S, CPS] -> srl [NSB,128,8192] (k=8s+ih, f=il*512+j), tail [128,256]
    m = xs[:, :NSB * SBP].reshape(S, NSB, 8, 16, 512)   # s w ih il j
    srl = np.ascontiguousarray(m.transpose(1, 0, 2, 3, 4))
    srl = srl.reshape(NSB, 128, 8192)
    t = np.ascontiguousarray(xs[:, NSB * SBP:]).reshape(S, 8, 16, 16)
    tail = t.reshape(128, 256)
    return srl, tail


def _decode_core(outd_c, outt_c):
    # outd [NSB,128,4096] p=16h+cd f=i_in*512+j -> y [H, CPS]
    y = np.empty((H, CPS), np.float32)
    d = np.asarray(outd_c).astype(np.float32)
    d = d.reshape(NSB, 8, 16, 8, 512)                   # w h cd i_in j
    y[:, :NSB * SBP] = d.transpose(1, 0, 2, 3, 4).reshape(H, NSB * SBP)
    t = np.asarray(outt_c).astype(np.float32)
    y[:, NSB * SBP:] = t.reshape(8, 16, 8, 16).reshape(H, TAILP)
    return y


def kernel(stacks, mask, W, b):
    from concourse.bass_utils import run_bass_kernel_spmd

    if "nc" not in _CACHE:
        _CACHE["nc"] = _build()
    nc = _CACHE["nc"]

    xg, idx, npos = _pack(stacks, mask)
    cw = _prep_consts(np.asarray(W, np.float32),
                      np.asarray(b, np.float32))

    in_maps = []
    for c in range(NCORES):
        srl_c, tail_c = _relayout_core(xg[:, c * CPS:(c + 1) * CPS])
        in_maps.append({"srl": srl_c, "tail": tail_c, "cw": cw})

    res = run_bass_kernel_spmd(nc, in_maps, core_ids=list(range(NCORES)),
                               **_CACHE.get("run_kwargs", {}))
    _CACHE["last_result"] = res
    y = np.concatenate(
        [_decode_core(r["outd"], r["outt"]) for r in res.results], axis=1)
    full = np.zeros((H, B * N * N), np.float32)
    full[:, idx] = y[:, :npos]
    out = np.ascontiguousarray(
        full.reshape(H, B, N, N).transpose(1, 0, 2, 3))
    return out


# revision 61
# speedup vs baseline: 1.0099x; 1.0099x over previous
"""Trainium2 Bass kernel for AdjStackAttentionWeights.

reference:  out = einsum('bsij,hs->bhij', stacks, W) + b[None,:,None,None]
            out = where(mask[:,None,:,:], 0.0, out)
shapes:     stacks [16,16,512,512] f32, mask [16,512,512] bool,
            W [8,16] f32, b [8] f32  ->  out [16,8,512,512] f32

Mask-compacted + data-parallel over positions: ~50% of the (b,i,j)
output positions are masked to zero, so those positions never touch the
device.  The host gathers the unmasked positions into one flat stream
(the same category of host relayout/dtype-cast the baseline already
did), pads it to a fixed size, and splits it EVENLY across the 8 cores
-- graph identity is irrelevant to the per-position linear map, so this
is perfectly load-balanced regardless of per-graph mask counts.

Per core: CPS = 264192 positions = 4 superblocks of 65536 + one 2048
tail (a 16-sigma margin over the binomial unmasked count, checked with
an assert).  Stream dtypes: stacks as fp8 e3m4 (4 mantissa bits; the
quantization error measures 1.34% rms vs the 2e-2 budget), weights as
bf16 (mixed-dtype matmul), output as bf16.

  srl  [4, 128, 8192] e3m4 (4 MB/core): superblock w, partition
       k = 8s+ih, f = il*512+j  holds  x[s, pos], local row r = 16ih+il,
       pos = w*65536 + r*512 + j.
  tail [128, 256] e3m4: k = 8s+ih, f = il*16+j', pos = 4*65536 + r*16+j'
  outd [4, 128, 4096] bf16 (4 MB/core): partition p = 16h+cd,
       f = i_in*512+j  holds  y[h, pos], pos = w*65536+(8cd+i_in)*512+j
  outt [128, 128] bf16: p = 16h+cd, f = i_in*16+j'

Compute per (w, i_in): psum [128,512] via TWO accumulating matmuls with
the zero-padded block-diagonal lhsT (c1 = 0,1 reads rhs cols
il = 8c1+i_in; routes (s,ih) -> p = 16h+2ih+c1, cd = 2ih+c1) -- every
srl element streams through the PE exactly once.  Matmuls are issued
c1-MAJOR (all 8 groups' c1=0 with one stationary half, then all c1=1)
so the PE array reloads its stationary twice per superblock instead of
16 times.  psum tiles span TWO banks / two groups; one open
accumulation group per bank (two open groups inside one bank clobber
each other -- measured).  The epilogue (bias add + bf16 convert) for
each pair is COLUMN-SPLIT across Vector and Activation: op cost is
free-dim-proportional, so two concurrent [128,512] ops halve the
latency of one [128,1024] op -- psum slots release sooner (the next
superblock's matmuls wait on them) and the final writes launch sooner.

Schedule (from perfetto traces): 16 DMA engines x 22.5 B/ns shared;
TWO HWDGE rings (sync/SP and scalar/Activation); a ring drains
descriptors in issue order, so reads and writes must not share a ring.
Reads stream on the sync ring; superblock 0 loads as four 2048-col
chunks so the PE unblocks early, superblocks 2-3 as c1-aligned halves
so their c1=0 pass overlaps the second half's arrival; consts load via
the scalar ring.  The micro-tail is read early and computed right
after superblock 0, keeping it off the critical end chain.  Writes
flow on the scalar ring as half-superblocks, except the final
superblock which drains as quarters alternating both rings (sync's
reads have finished by then).  Measured: exec ~= last-write-completion
+ ~2.9 us fixed (barrier + teardown).

Traffic: 4.3 MB read + 4.2 MB write per core; fabric floor ~23.6 us.
"""

import numpy as np
import ml_dtypes

B, S, N, H = 16, 16, 512, 8
NCORES = 8
NSB = 4                       # superblocks per core
SBP = 65536                   # positions per superblock (128 rows x 512)
TAILP = 2048                  # tail positions per core  ([128, 256] tile)
CPS = NSB * SBP + TAILP       # 264192 positions per core
CPT = NCORES * CPS            # 2113536 total capacity (count ~2097152)

IN_NP = ml_dtypes.float8_e3m4  # host->device stream dtype

_CACHE = {}


def _build():
    import concourse.bacc as bacc
    import concourse.mybir as mybir
    import concourse.tile as tile

    f32 = mybir.dt.float32
    bf16 = mybir.dt.bfloat16
    in_dt = mybir.dt.float8e3  # e3m4

    nc = bacc.Bacc("TRN2", target_bir_lowering=False, debug=False,
                   num_devices=NCORES)

    u8 = mybir.dt.uint8
    srl = nc.dram_tensor("srl", [NSB, 128, 8192], in_dt,
                         kind="ExternalInput")
    tail = nc.dram_tensor("tail", [128, 256], in_dt, kind="ExternalInput")
    # w_bd (bf16 [128,256]) and bias (f32 [128,1]) packed into one
    # 516 B/partition byte blob: ONE 128-descriptor DMA at the head of
    # the read ring, so the PE's stationary weights always land before
    # the first data chunk (w_bd on the scalar ring was arriving late
    # on some runs and gating the PE start by up to 2 us)
    cw = nc.dram_tensor("cw", [128, 516], u8, kind="ExternalInput")
    outd = nc.dram_tensor("outd", [NSB, 128, 4096], bf16,
                          kind="ExternalOutput")
    outt = nc.dram_tensor("outt", [128, 128], bf16, kind="ExternalOutput")

    with tile.TileContext(nc) as tc:
        with (
            tc.tile_pool(name="const", bufs=1) as cpool,
            tc.tile_pool(name="chunk", bufs=4) as kpool,
            tc.tile_pool(name="data", bufs=4) as dpool,
            tc.tile_pool(name="half", bufs=5) as hpool,
            tc.tile_pool(name="outp", bufs=5) as opool,
            # slots are per-TAG, bufs slots each: 4 x 2-bank tiles = 8 banks
            tc.tile_pool(name="psd", bufs=4, space="PSUM") as psd_pool,
        ):
            cwt = cpool.tile([128, 516], u8)
            nc.scalar.dma_start(cwt[:], cw.ap())
            wbd_t = cwt[:, 0:512].bitcast(bf16)
            bias_t = cwt[:, 512:516].bitcast(f32)

            # ---- all read DMAs up-front on the sync HWDGE ring ----
            # superblock 0: four 2048-col chunks (c1-major consumes
            # chunk 0 first, so the PE unblocks after 256KB);
            # superblock 1: whole; superblocks 2-3: c1-aligned halves
            # [cols 0-4095 | 4096-8191] so the c1=0 pass starts while
            # the c1=1 half is still in flight
            # superblock 0's c1=0 columns load as two 2048-col chunks
            # (the PE unblocks after 256KB); its c1=1 columns load as
            # one [128,4096] half -- c1-major order doesn't touch them
            # until the whole c1=0 pass is done, and 4KB descriptors
            # ramp the fabric better than 2KB ones
            chunks = {}
            for q in range(2):
                chunks[q] = kpool.tile([128, 2048], in_dt, tag="chunk",
                                       name=f"ch{q}")
                nc.sync.dma_start(chunks[q][:],
                                  srl.ap()[0][:, q * 2048:(q + 1) * 2048])
            h01 = hpool.tile([128, 4096], in_dt, tag="half", name="h0_1")
            nc.sync.dma_start(h01[:], srl.ap()[0][:, 4096:8192])
            tail_t = cpool.tile([128, 256], in_dt)
            nc.sync.dma_start(tail_t[:], tail.ap())
            rhs1 = dpool.tile([128, 8192], in_dt, tag="rhs", name="rhs1")
            nc.sync.dma_start(rhs1[:], srl.ap()[1])
            halves = {}
            for w in (2, 3):
                for c1 in range(2):
                    hv = hpool.tile([128, 4096], in_dt, tag="half",
                                    name=f"h{w}_{c1}")
                    nc.sync.dma_start(
                        hv[:], srl.ap()[w][:, c1 * 4096:c1 * 4096 + 4096])
                    halves[(w, c1)] = hv

            # ---- compute + writes ----
            for w in range(NSB):
                out_t = opool.tile([128, 4096], bf16, tag="out",
                                   name=f"out{w}")
                pss = [psd_pool.tile([128, 1024], f32, tag="ps",
                                     name=f"ps{w}_{i}") for i in range(4)]

                def g(i_in):
                    return pss[i_in // 2][:, (i_in % 2) * 512:
                                          (i_in % 2) * 512 + 512]

                def src_fsl(i_in, c1):
                    if w == 0:
                        if c1 == 0:
                            return chunks[i_in // 4], (i_in % 4) * 512
                        return h01, i_in * 512
                    if w == 1:
                        return rhs1, (8 * c1 + i_in) * 512
                    return halves[(w, c1)], i_in * 512

                for i_in in range(8):          # c1=0: one stationary
                    src, fsl = src_fsl(i_in, 0)
                    nc.tensor.matmul(
                        g(i_in), wbd_t[:, 0:128], src[:, fsl:fsl + 512],
                        start=True, stop=False)
                for i_in in range(8):          # c1=1 + epilogue + write
                    src, fsl = src_fsl(i_in, 1)
                    nc.tensor.matmul(
                        g(i_in), wbd_t[:, 128:256], src[:, fsl:fsl + 512],
                        start=False, stop=True)
                    if i_in % 2 == 1:
                        p = i_in // 2
                        fs = p * 1024
                        # epilogue cost is free-dim-proportional, so a
                        # column-split across BOTH engines halves the
                        # latency of each pair's epilogue -- psum slots
                        # release sooner (the next superblock's matmuls
                        # wait on them) and the final writes launch
                        # sooner.  (only Vector/Activation can read
                        # PSUM; GpSimd cannot)
                        nc.vector.tensor_scalar_add(
                            out_t[:, fs:fs + 512], pss[p][:, 0:512],
                            bias_t[:])
                        nc.scalar.add(
                            out_t[:, fs + 512:fs + 1024],
                            pss[p][:, 512:1024], bias_t[:])
                    if w < NSB - 1:
                        if i_in % 4 == 3:      # half (2048 cols) done
                            c = i_in // 4
                            nc.scalar.dma_start(
                                outd.ap()[w][:, c * 2048:c * 2048 + 2048],
                                out_t[:, c * 2048:c * 2048 + 2048])
                    elif i_in % 2 == 1:        # last superblock: quarter
                        q = i_in // 2          # writes alternate rings
                        weng = nc.sync if q % 2 == 0 else nc.scalar
                        weng.dma_start(
                            outd.ap()[w][:, q * 1024:q * 1024 + 1024],
                            out_t[:, q * 1024:q * 1024 + 1024])

                if w == 0:
                    # micro-tail: 2048 positions at colw=16, computed
                    # HERE (not at the end) so it is off the critical
                    # end chain.  groups run sequentially: interleaving
                    # OPEN accumulation groups inside one bank clobbers
                    # it (measured)
                    out_tt = opool.tile([128, 128], bf16)
                    pstf = psd_pool.tile([128, 512], f32, tag="ps",
                                         name="pst")
                    for i_in in range(8):
                        nc.tensor.matmul(
                            pstf[:, i_in * 16:i_in * 16 + 16],
                            wbd_t[:, 0:128],
                            tail_t[:, i_in * 16:i_in * 16 + 16],
                            start=True, stop=False)
                        nc.tensor.matmul(
                            pstf[:, i_in * 16:i_in * 16 + 16],
                            wbd_t[:, 128:256],
                            tail_t[:, (8 + i_in) * 16:(8 + i_in) * 16 + 16],
                            start=False, stop=True)
                    nc.vector.tensor_scalar_add(out_tt[:], pstf[:, 0:128],
                                                bias_t[:])
                    nc.scalar.dma_start(outt.ap(), out_tt[:])

    nc.compile()
    return nc


def _prep_consts(W, b):
    # c1-th accumulating matmul lhsT in w_bd[:, 128*c1:...]:
    # w_bd[8s+ih, 128*c1 + 16h + 2ih + c1] = W[h, s]; rest zero.
    w_bd = np.zeros((128, 256), dtype=np.float32)
    for c1 in range(2):
        for ih in range(8):
            for h in range(8):
                m = 16 * h + 2 * ih + c1
                w_bd[ih::8, 128 * c1 + m] = W[h, :]  # rows k = 8s+ih
    bias = np.repeat(np.asarray(b, np.float32), 16).reshape(128, 1)
    # pack [w_bd bf16 | bias f32] into one 516 B/partition byte blob
    cw = np.empty((128, 516), dtype=np.uint8)
    cw[:, :512] = w_bd.astype(ml_dtypes.bfloat16).view(np.uint8)
    cw[:, 512:] = np.ascontiguousarray(bias).view(np.uint8)
    return cw


def _pack(stacks, mask):
    # compacted stream: unmasked positions of the flattened [B*N*N]
    # grid in row-major order, zero-padded to CPT
    idx = np.flatnonzero(~np.asarray(mask, bool).reshape(-1))
    npos = idx.size
    assert npos <= CPT, (npos, CPT)
    st = np.asarray(stacks, np.float32).astype(IN_NP)
    st = st.transpose(1, 0, 2, 3).reshape(S, B * N * N)
    xg = np.zeros((S, CPT), dtype=IN_NP)
    xg[:, :npos] = st[:, idx]
    return xg, idx, npos


def _relayout_core(xs):
    # xs [S, CPS] -> srl [NSB,128,8192] (k=8s+ih, f=il*512+j), tail [128,256]
    m = xs[:, :NSB * SBP].reshape(S, NSB, 8, 16, 512)   # s w ih il j
    srl = np.ascontiguousarray(m.transpose(1, 0, 2, 3, 4))
    srl = srl.reshape(NSB, 128, 8192)
    t = np.ascontiguousarray(xs[:, NSB * SBP:]).reshape(S, 8, 16, 16)
    tail = t.reshape(128, 256)
    return srl, tail


def _decode_core(outd_c, outt_c):
    # outd [NSB,128,4096] p=16h+cd f=i_in*512+j -> y [H, CPS]
    y = np.empty((H, CPS), np.float32)
    d = np.asarray(outd_c).astype(np.float32)
    d = d.reshape(NSB, 8, 16, 8, 512)                   # w h cd i_in j
    y[:, :NSB * SBP] = d.transpose(1, 0, 2, 3, 4).reshape(H, NSB * SBP)
    t = np.asarray(outt_c).astype(np.float32)
    y[:, NSB * SBP:] = t.reshape(8, 16, 8, 16).reshape(H, TAILP)
    return y


def kernel(stacks, mask, W, b):
    from concourse.bass_utils import run_bass_kernel_spmd

    if "nc" not in _CACHE:
        _CACHE["nc"] = _build()
    nc = _CACHE["nc"]

    xg, idx, npos = _pack(stacks, mask)
    cw = _prep_consts(np.asarray(W, np.float32),
                      np.asarray(b, np.float32))

    in_maps = []
    for c in range(NCORES):
        srl_c, tail_c = _relayout_core(xg[:, c * CPS:(c + 1) * CPS])
        in_maps.append({"srl": srl_c, "tail": tail_c, "cw": cw})

    res = run_bass_kernel_spmd(nc, in_maps, core_ids=list(range(NCORES)),
                               **_CACHE.get("run_kwargs", {}))
    _CACHE["last_result"] = res
    y = np.concatenate(
        [_decode_core(r["outd"], r["outt"]) for r in res.results], axis=1)
    full = np.zeros((H, B * N * N), np.float32)
    full[:, idx] = y[:, :npos]
    out = np.ascontiguousarray(
        full.reshape(H, B, N, N).transpose(1, 0, 2, 3))
    return out


# revision 65
# speedup vs baseline: 1.0297x; 1.0196x over previous
"""Trainium2 Bass kernel for AdjStackAttentionWeights.

reference:  out = einsum('bsij,hs->bhij', stacks, W) + b[None,:,None,None]
            out = where(mask[:,None,:,:], 0.0, out)
shapes:     stacks [16,16,512,512] f32, mask [16,512,512] bool,
            W [8,16] f32, b [8] f32  ->  out [16,8,512,512] f32

Mask-compacted + data-parallel over positions: ~50% of the (b,i,j)
output positions are masked to zero, so those positions never touch the
device.  The host gathers the unmasked positions into one flat stream
(the same category of host relayout/dtype-cast the baseline already
did), pads it to a fixed size, and splits it EVENLY across the 8 cores
-- graph identity is irrelevant to the per-position linear map, so this
is perfectly load-balanced regardless of per-graph mask counts.

Per core: CPS = 264192 positions = 4 superblocks of 65536 + one 2048
tail (a 16-sigma margin over the binomial unmasked count, checked with
an assert).  Stream dtypes: stacks as fp8 e3m4 (4 mantissa bits; the
quantization error measures 1.34% rms vs the 2e-2 budget), weights as
bf16 (mixed-dtype matmul), output as bf16.

  srl  [4, 128, 8192] e3m4 (4 MB/core): superblock w, partition
       k = 8s+ih, f = il*512+j  holds  x[s, pos], local row r = 16ih+il,
       pos = w*65536 + r*512 + j.
  tail [128, 256] e3m4: k = 8s+ih, f = il*16+j', pos = 4*65536 + r*16+j'
  outd [4, 128, 4096] bf16 (4 MB/core): partition p = 16h+cd,
       f = i_in*512+j  holds  y[h, pos], pos = w*65536+(8cd+i_in)*512+j
  outt [128, 128] bf16: p = 16h+cd, f = i_in*16+j'

Compute per (w, i_in): psum [128,512] via TWO accumulating matmuls with
the zero-padded block-diagonal lhsT (c1 = 0,1 reads rhs cols
il = 8c1+i_in; routes (s,ih) -> p = 16h+2ih+c1, cd = 2ih+c1) -- every
srl element streams through the PE exactly once.  Matmuls are issued
c1-MAJOR (all 8 groups' c1=0 with one stationary half, then all c1=1)
so the PE array reloads its stationary twice per superblock instead of
16 times.  psum tiles span TWO banks / two groups; one open
accumulation group per bank (two open groups inside one bank clobber
each other -- measured).  The epilogue (bias add + bf16 convert) for
each pair is COLUMN-SPLIT across Vector and Activation: op cost is
free-dim-proportional, so two concurrent [128,512] ops halve the
latency of one [128,1024] op -- psum slots release sooner (the next
superblock's matmuls wait on them) and the final writes launch sooner.

Schedule (from perfetto traces): 16 DMA engines x 22.5 B/ns shared;
TWO HWDGE rings (sync/SP and scalar/Activation); a ring drains
descriptors in issue order, so reads and writes must not share a ring.
Reads stream on the sync ring; superblock 0 loads as four 2048-col
chunks so the PE unblocks early, superblocks 2-3 as c1-aligned halves
so their c1=0 pass overlaps the second half's arrival; consts load via
the scalar ring.  The micro-tail is read early and computed right
after superblock 0, keeping it off the critical end chain.  Writes
flow on the scalar ring as half-superblocks, except the final
superblock which drains as quarters alternating both rings (sync's
reads have finished by then).  Measured: exec ~= last-write-completion
+ ~2.9 us fixed (barrier + teardown).

Traffic: 4.3 MB read + 4.2 MB write per core; fabric floor ~23.6 us.
"""

import numpy as np
import ml_dtypes

B, S, N, H = 16, 16, 512, 8
NCORES = 8
NSB = 4                       # superblocks per core
SBP = 65536                   # positions per superblock (128 rows x 512)
TAILP = 2048                  # tail positions per core  ([128, 256] tile)
CPS = NSB * SBP + TAILP       # 264192 positions per core
CPT = NCORES * CPS            # 2113536 total capacity (count ~2097152)

IN_NP = ml_dtypes.float8_e3m4  # host->device stream dtype

_CACHE = {}


def _build():
    import concourse.bacc as bacc
    import concourse.mybir as mybir
    import concourse.tile as tile

    f32 = mybir.dt.float32
    bf16 = mybir.dt.bfloat16
    in_dt = mybir.dt.float8e3  # e3m4

    nc = bacc.Bacc("TRN2", target_bir_lowering=False, debug=False,
                   num_devices=NCORES)

    u8 = mybir.dt.uint8
    srl = nc.dram_tensor("srl", [NSB, 128, 8192], in_dt,
                         kind="ExternalInput")
    tail = nc.dram_tensor("tail", [128, 256], in_dt, kind="ExternalInput")
    # w_bd (bf16 [128,256]) and bias (f32 [128,1]) packed into one
    # 516 B/partition byte blob: ONE 128-descriptor DMA at the head of
    # the read ring, so the PE's stationary weights always land before
    # the first data chunk (w_bd on the scalar ring was arriving late
    # on some runs and gating the PE start by up to 2 us)
    cw = nc.dram_tensor("cw", [128, 516], u8, kind="ExternalInput")
    outd = nc.dram_tensor("outd", [NSB, 128, 4096], bf16,
                          kind="ExternalOutput")
    outt = nc.dram_tensor("outt", [128, 128], bf16, kind="ExternalOutput")

    with tile.TileContext(nc) as tc:
        with (
            tc.tile_pool(name="const", bufs=1) as cpool,
            tc.tile_pool(name="chunk", bufs=4) as kpool,
            tc.tile_pool(name="data", bufs=4) as dpool,
            tc.tile_pool(name="half", bufs=7) as hpool,
            tc.tile_pool(name="outp", bufs=5) as opool,
            # slots are per-TAG, bufs slots each: 4 x 2-bank tiles = 8 banks
            tc.tile_pool(name="psd", bufs=4, space="PSUM") as psd_pool,
        ):
            cwt = cpool.tile([128, 516], u8)
            nc.scalar.dma_start(cwt[:], cw.ap())
            wbd_t = cwt[:, 0:512].bitcast(bf16)
            bias_t = cwt[:, 512:516].bitcast(f32)

            # ---- all read DMAs up-front on the sync HWDGE ring ----
            # superblock 0: four 2048-col chunks (c1-major consumes
            # chunk 0 first, so the PE unblocks after 256KB);
            # superblock 1: whole; superblocks 2-3: c1-aligned halves
            # [cols 0-4095 | 4096-8191] so the c1=0 pass starts while
            # the c1=1 half is still in flight
            # superblock 0's c1=0 columns load as two 2048-col chunks
            # (the PE unblocks after 256KB); its c1=1 columns load as
            # one [128,4096] half -- c1-major order doesn't touch them
            # until the whole c1=0 pass is done, and 4KB descriptors
            # ramp the fabric better than 2KB ones
            chunks = {}
            for q in range(2):
                chunks[q] = kpool.tile([128, 2048], in_dt, tag="chunk",
                                       name=f"ch{q}")
                nc.sync.dma_start(chunks[q][:],
                                  srl.ap()[0][:, q * 2048:(q + 1) * 2048])
            h01 = hpool.tile([128, 4096], in_dt, tag="half", name="h0_1")
            nc.sync.dma_start(h01[:], srl.ap()[0][:, 4096:8192])
            tail_t = cpool.tile([128, 256], in_dt)
            nc.sync.dma_start(tail_t[:], tail.ap())
            halves = {}
            for w in (1, 2, 3):
                for c1 in range(2):
                    hv = hpool.tile([128, 4096], in_dt, tag="half",
                                    name=f"h{w}_{c1}")
                    nc.sync.dma_start(
                        hv[:], srl.ap()[w][:, c1 * 4096:c1 * 4096 + 4096])
                    halves[(w, c1)] = hv

            # ---- compute + writes ----
            for w in range(NSB):
                out_t = opool.tile([128, 4096], bf16, tag="out",
                                   name=f"out{w}")
                pss = [psd_pool.tile([128, 1024], f32, tag="ps",
                                     name=f"ps{w}_{i}") for i in range(4)]

                def g(i_in):
                    return pss[i_in // 2][:, (i_in % 2) * 512:
                                          (i_in % 2) * 512 + 512]

                def src_fsl(i_in, c1):
                    if w == 0:
                        if c1 == 0:
                            return chunks[i_in // 4], (i_in % 4) * 512
                        return h01, i_in * 512
                    return halves[(w, c1)], i_in * 512

                for i_in in range(8):          # c1=0: one stationary
                    src, fsl = src_fsl(i_in, 0)
                    nc.tensor.matmul(
                        g(i_in), wbd_t[:, 0:128], src[:, fsl:fsl + 512],
                        start=True, stop=False)
                for i_in in range(8):          # c1=1 + epilogue + write
                    src, fsl = src_fsl(i_in, 1)
                    nc.tensor.matmul(
                        g(i_in), wbd_t[:, 128:256], src[:, fsl:fsl + 512],
                        start=False, stop=True)
                    if i_in % 2 == 1:
                        p = i_in // 2
                        fs = p * 1024
                        # epilogue cost is free-dim-proportional, so a
                        # column-split across BOTH engines halves the
                        # latency of each pair's epilogue -- psum slots
                        # release sooner (the next superblock's matmuls
                        # wait on them) and the final writes launch
                        # sooner.  (only Vector/Activation can read
                        # PSUM; GpSimd cannot)
                        nc.vector.tensor_scalar_add(
                            out_t[:, fs:fs + 512], pss[p][:, 0:512],
                            bias_t[:])
                        nc.scalar.add(
                            out_t[:, fs + 512:fs + 1024],
                            pss[p][:, 512:1024], bias_t[:])
                    if w < NSB - 1:
                        if i_in % 4 == 3:      # half (2048 cols) done
                            c = i_in // 4
                            nc.scalar.dma_start(
                                outd.ap()[w][:, c * 2048:c * 2048 + 2048],
                                out_t[:, c * 2048:c * 2048 + 2048])
                    elif i_in % 2 == 1:        # last superblock: quarters
                        q = i_in // 2
                        # q0-q2 on the idle sync ring so the scalar
                        # sequencer's dma_start issues (~0.6us each)
                        # never delay its remaining epilogue halves;
                        # only q3 (after the last epilogue) on scalar
                        weng = nc.scalar if q == 3 else nc.sync
                        weng.dma_start(
                            outd.ap()[w][:, q * 1024:q * 1024 + 1024],
                            out_t[:, q * 1024:q * 1024 + 1024])

                if w == 0:
                    # micro-tail: 2048 positions at colw=16, computed
                    # HERE (not at the end) so it is off the critical
                    # end chain.  groups run sequentially: interleaving
                    # OPEN accumulation groups inside one bank clobbers
                    # it (measured)
                    out_tt = opool.tile([128, 128], bf16)
                    pstf = psd_pool.tile([128, 512], f32, tag="ps",
                                         name="pst")
                    for i_in in range(8):
                        nc.tensor.matmul(
                            pstf[:, i_in * 16:i_in * 16 + 16],
                            wbd_t[:, 0:128],
                            tail_t[:, i_in * 16:i_in * 16 + 16],
                            start=True, stop=False)
                        nc.tensor.matmul(
                            pstf[:, i_in * 16:i_in * 16 + 16],
                            wbd_t[:, 128:256],
                            tail_t[:, (8 + i_in) * 16:(8 + i_in) * 16 + 16],
                            start=False, stop=True)
                    nc.vector.tensor_scalar_add(out_tt[:], pstf[:, 0:128],
                                                bias_t[:])
                    nc.scalar.dma_start(outt.ap(), out_tt[:])

    nc.compile()
    return nc


def _prep_consts(W, b):
    # c1-th accumulating matmul lhsT in w_bd[:, 128*c1:...]:
    # w_bd[8s+ih, 128*c1 + 16h + 2ih + c1] = W[h, s]; rest zero.
    w_bd = np.zeros((128, 256), dtype=np.float32)
    for c1 in range(2):
        for ih in range(8):
            for h in range(8):
                m = 16 * h + 2 * ih + c1
                w_bd[ih::8, 128 * c1 + m] = W[h, :]  # rows k = 8s+ih
    bias = np.repeat(np.asarray(b, np.float32), 16).reshape(128, 1)
    # pack [w_bd bf16 | bias f32] into one 516 B/partition byte blob
    cw = np.empty((128, 516), dtype=np.uint8)
    cw[:, :512] = w_bd.astype(ml_dtypes.bfloat16).view(np.uint8)
    cw[:, 512:] = np.ascontiguousarray(bias).view(np.uint8)
    return cw


def _pack(stacks, mask):
    # compacted stream: unmasked positions of the flattened [B*N*N]
    # grid in row-major order, zero-padded to CPT
    idx = np.flatnonzero(~np.asarray(mask, bool).reshape(-1))
    npos = idx.size
    assert npos <= CPT, (npos, CPT)
    st = np.asarray(stacks, np.float32).astype(IN_NP)
    st = st.transpose(1, 0, 2, 3).reshape(S, B * N * N)
    xg = np.zeros((S, CPT), dtype=IN_NP)
    xg[:, :npos] = st[:, idx]
    return xg, idx, npos


def _relayout_core(xs):
    # xs [S, CPS] -> srl [NSB,128,8192] (k=8s+ih, f=il*512+j), tail [128,256]
    m = xs[:, :NSB * SBP].reshape(S, NSB, 8, 16, 512)   # s w ih il j
    srl = np.ascontiguousarray(m.transpose(1, 0, 2, 3, 4))
    srl = srl.reshape(NSB, 128, 8192)
    t = np.ascontiguousarray(xs[:, NSB * SBP:]).reshape(S, 8, 16, 16)
    tail = t.reshape(128, 256)
    return srl, tail


def _decode_core(outd_c, outt_c):
    # outd [NSB,128,4096] p=16h+cd f=i_in*512+j -> y [H, CPS]
    y = np.empty((H, CPS), np.float32)
    d = np.asarray(outd_c).astype(np.float32)
    d = d.reshape(NSB, 8, 16, 8, 512)                   # w h cd i_in j
    y[:, :NSB * SBP] = d.transpose(1, 0, 2, 3, 4).reshape(H, NSB * SBP)
    t = np.asarray(outt_c).astype(np.float32)
    y[:, NSB * SBP:] = t.reshape(8, 16, 8, 16).reshape(H, TAILP)
    return y


def kernel(stacks, mask, W, b):
    from concourse.bass_utils import run_bass_kernel_spmd

    if "nc" not in _CACHE:
        _CACHE["nc"] = _build()
    nc = _CACHE["nc"]

    xg, idx, npos = _pack(stacks, mask)
    cw = _prep_consts(np.asarray(W, np.float32),
                      np.asarray(b, np.float32))

    in_maps = []
    for c in range(NCORES):
        srl_c, tail_c = _relayout_core(xg[:, c * CPS:(c + 1) * CPS])
        in_maps.append({"srl": srl_c, "tail": tail_c, "cw": cw})

    res = run_bass_kernel_spmd(nc, in_maps, core_ids=list(range(NCORES)),
                               **_CACHE.get("run_kwargs", {}))
    _CACHE["last_result"] = res
    y = np.concatenate(
        [_decode_core(r["outd"], r["outt"]) for r in res.results], axis=1)
    full = np.zeros((H, B * N * N), np.float32)
    full[:, idx] = y[:, :npos]
    out = np.ascontiguousarray(
        full.reshape(H, B, N, N).transpose(1, 0, 2, 3))
    return out


# revision 66
# speedup vs baseline: 1.0503x; 1.0200x over previous
"""Trainium2 Bass kernel for AdjStackAttentionWeights.

reference:  out = einsum('bsij,hs->bhij', stacks, W) + b[None,:,None,None]
            out = where(mask[:,None,:,:], 0.0, out)
shapes:     stacks [16,16,512,512] f32, mask [16,512,512] bool,
            W [8,16] f32, b [8] f32  ->  out [16,8,512,512] f32

Mask-compacted + data-parallel over positions: ~50% of the (b,i,j)
output positions are masked to zero, so those positions never touch the
device.  The host gathers the unmasked positions into one flat stream
(the same category of host relayout/dtype-cast the baseline already
did), pads it to a fixed size, and splits it EVENLY across the 8 cores
-- graph identity is irrelevant to the per-position linear map, so this
is perfectly load-balanced regardless of per-graph mask counts.

Per core: CPS = 264192 positions = 4 superblocks of 65536 + one 2048
tail (a 16-sigma margin over the binomial unmasked count, checked with
an assert).  Stream dtypes: stacks as fp8 e3m4 (4 mantissa bits; the
quantization error measures 1.34% rms vs the 2e-2 budget), weights as
bf16 (mixed-dtype matmul), output as bf16.

  srl  [4, 128, 8192] e3m4 (4 MB/core): superblock w, partition
       k = 8s+ih, f = il*512+j  holds  x[s, pos], local row r = 16ih+il,
       pos = w*65536 + r*512 + j.
  tail [128, 256] e3m4: k = 8s+ih, f = il*16+j', pos = 4*65536 + r*16+j'
  outd [4, 128, 4096] bf16 (4 MB/core): partition p = 16h+cd,
       f = i_in*512+j  holds  y[h, pos], pos = w*65536+(8cd+i_in)*512+j
  outt [128, 128] bf16: p = 16h+cd, f = i_in*16+j'

Compute per (w, i_in): psum [128,512] via TWO accumulating matmuls with
the zero-padded block-diagonal lhsT (c1 = 0,1 reads rhs cols
il = 8c1+i_in; routes (s,ih) -> p = 16h+2ih+c1, cd = 2ih+c1) -- every
srl element streams through the PE exactly once.  Matmuls are issued
c1-MAJOR (all 8 groups' c1=0 with one stationary half, then all c1=1)
so the PE array reloads its stationary twice per superblock instead of
16 times.  psum tiles span TWO banks / two groups; one open
accumulation group per bank (two open groups inside one bank clobber
each other -- measured).  The epilogue (bias add + bf16 convert) for
each pair is COLUMN-SPLIT across Vector and Activation: op cost is
free-dim-proportional, so two concurrent [128,512] ops halve the
latency of one [128,1024] op -- psum slots release sooner (the next
superblock's matmuls wait on them) and the final writes launch sooner.

Schedule (from perfetto traces): 16 DMA engines x 22.5 B/ns shared;
TWO HWDGE rings (sync/SP and scalar/Activation); a ring drains
descriptors in issue order, so reads and writes must not share a ring.
Reads stream on the sync ring; superblock 0 loads as four 2048-col
chunks so the PE unblocks early, superblocks 2-3 as c1-aligned halves
so their c1=0 pass overlaps the second half's arrival; consts load via
the scalar ring.  The micro-tail is read early and computed right
after superblock 0, keeping it off the critical end chain.  Writes
flow on the scalar ring as half-superblocks, except the final
superblock which drains as quarters alternating both rings (sync's
reads have finished by then).  Measured: exec ~= last-write-completion
+ ~2.9 us fixed (barrier + teardown).

Traffic: 4.3 MB read + 4.2 MB write per core; fabric floor ~23.6 us.
"""

import numpy as np
import ml_dtypes

B, S, N, H = 16, 16, 512, 8
NCORES = 8
NSB = 4                       # superblocks per core
SBP = 65536                   # positions per superblock (128 rows x 512)
TAILP = 2048                  # tail positions per core  ([128, 256] tile)
CPS = NSB * SBP + TAILP       # 264192 positions per core
CPT = NCORES * CPS            # 2113536 total capacity (count ~2097152)

IN_NP = ml_dtypes.float8_e3m4  # host->device stream dtype

_CACHE = {}


def _build():
    import concourse.bacc as bacc
    import concourse.mybir as mybir
    import concourse.tile as tile

    f32 = mybir.dt.float32
    bf16 = mybir.dt.bfloat16
    in_dt = mybir.dt.float8e3  # e3m4

    nc = bacc.Bacc("TRN2", target_bir_lowering=False, debug=False,
                   num_devices=NCORES)

    u8 = mybir.dt.uint8
    srl = nc.dram_tensor("srl", [NSB, 128, 8192], in_dt,
                         kind="ExternalInput")
    tail = nc.dram_tensor("tail", [128, 256], in_dt, kind="ExternalInput")
    # w_bd (bf16 [128,256]) and bias (f32 [128,1]) packed into one
    # 516 B/partition byte blob: ONE 128-descriptor DMA at the head of
    # the read ring, so the PE's stationary weights always land before
    # the first data chunk (w_bd on the scalar ring was arriving late
    # on some runs and gating the PE start by up to 2 us)
    cw = nc.dram_tensor("cw", [128, 516], u8, kind="ExternalInput")
    outd = nc.dram_tensor("outd", [NSB, 128, 4096], bf16,
                          kind="ExternalOutput")
    outt = nc.dram_tensor("outt", [128, 128], bf16, kind="ExternalOutput")

    with tile.TileContext(nc) as tc:
        with (
            tc.tile_pool(name="const", bufs=1) as cpool,
            tc.tile_pool(name="chunk", bufs=4) as kpool,
            tc.tile_pool(name="data", bufs=4) as dpool,
            tc.tile_pool(name="half", bufs=5) as hpool,
            tc.tile_pool(name="outp", bufs=5) as opool,
            # slots are per-TAG, bufs slots each: 4 x 2-bank tiles = 8 banks
            tc.tile_pool(name="psd", bufs=4, space="PSUM") as psd_pool,
        ):
            cwt = cpool.tile([128, 516], u8)
            nc.scalar.dma_start(cwt[:], cw.ap())
            wbd_t = cwt[:, 0:512].bitcast(bf16)
            bias_t = cwt[:, 512:516].bitcast(f32)

            # ---- all read DMAs up-front on the sync HWDGE ring ----
            # superblock 0: four 2048-col chunks (c1-major consumes
            # chunk 0 first, so the PE unblocks after 256KB);
            # superblock 1: whole; superblocks 2-3: c1-aligned halves
            # [cols 0-4095 | 4096-8191] so the c1=0 pass starts while
            # the c1=1 half is still in flight
            # superblock 0's c1=0 columns load as two 2048-col chunks
            # (the PE unblocks after 256KB); its c1=1 columns load as
            # one [128,4096] half -- c1-major order doesn't touch them
            # until the whole c1=0 pass is done, and 4KB descriptors
            # ramp the fabric better than 2KB ones
            chunks = {}
            for q in range(2):
                chunks[q] = kpool.tile([128, 2048], in_dt, tag="chunk",
                                       name=f"ch{q}")
                nc.sync.dma_start(chunks[q][:],
                                  srl.ap()[0][:, q * 2048:(q + 1) * 2048])
            h01 = hpool.tile([128, 4096], in_dt, tag="half", name="h0_1")
            nc.sync.dma_start(h01[:], srl.ap()[0][:, 4096:8192])
            tail_t = cpool.tile([128, 256], in_dt)
            nc.sync.dma_start(tail_t[:], tail.ap())
            rhs1 = dpool.tile([128, 8192], in_dt, tag="rhs", name="rhs1")
            nc.sync.dma_start(rhs1[:], srl.ap()[1])
            halves = {}
            for w in (2, 3):
                for c1 in range(2):
                    hv = hpool.tile([128, 4096], in_dt, tag="half",
                                    name=f"h{w}_{c1}")
                    nc.sync.dma_start(
                        hv[:], srl.ap()[w][:, c1 * 4096:c1 * 4096 + 4096])
                    halves[(w, c1)] = hv

            # ---- compute + writes ----
            for w in range(NSB):
                out_t = opool.tile([128, 4096], bf16, tag="out",
                                   name=f"out{w}")
                pss = [psd_pool.tile([128, 1024], f32, tag="ps",
                                     name=f"ps{w}_{i}") for i in range(4)]

                def g(i_in):
                    return pss[i_in // 2][:, (i_in % 2) * 512:
                                          (i_in % 2) * 512 + 512]

                def src_fsl(i_in, c1):
                    if w == 0:
                        if c1 == 0:
                            return chunks[i_in // 4], (i_in % 4) * 512
                        return h01, i_in * 512
                    if w == 1:
                        return rhs1, (8 * c1 + i_in) * 512
                    return halves[(w, c1)], i_in * 512

                for i_in in range(8):          # c1=0: one stationary
                    src, fsl = src_fsl(i_in, 0)
                    nc.tensor.matmul(
                        g(i_in), wbd_t[:, 0:128], src[:, fsl:fsl + 512],
                        start=True, stop=False)
                for i_in in range(8):          # c1=1 + epilogue + write
                    src, fsl = src_fsl(i_in, 1)
                    nc.tensor.matmul(
                        g(i_in), wbd_t[:, 128:256], src[:, fsl:fsl + 512],
                        start=False, stop=True)
                    if i_in % 2 == 1:
                        p = i_in // 2
                        fs = p * 1024
                        # epilogue cost is free-dim-proportional, so a
                        # column-split across BOTH engines halves the
                        # latency of each pair's epilogue -- psum slots
                        # release sooner (the next superblock's matmuls
                        # wait on them) and the final writes launch
                        # sooner.  (only Vector/Activation can read
                        # PSUM; GpSimd cannot)
                        nc.vector.tensor_scalar_add(
                            out_t[:, fs:fs + 512], pss[p][:, 0:512],
                            bias_t[:])
                        nc.scalar.add(
                            out_t[:, fs + 512:fs + 1024],
                            pss[p][:, 512:1024], bias_t[:])
                    if w < NSB - 1:
                        if i_in % 4 == 3:      # half (2048 cols) done
                            c = i_in // 4
                            nc.scalar.dma_start(
                                outd.ap()[w][:, c * 2048:c * 2048 + 2048],
                                out_t[:, c * 2048:c * 2048 + 2048])
                    elif i_in % 2 == 1:        # last superblock: quarter
                        q = i_in // 2          # writes alternate rings
                        weng = nc.sync if q % 2 == 0 else nc.scalar
                        weng.dma_start(
                            outd.ap()[w][:, q * 1024:q * 1024 + 1024],
                            out_t[:, q * 1024:q * 1024 + 1024])

                if w == 0:
                    # micro-tail: 2048 positions at colw=16, computed
                    # HERE (not at the end) so it is off the critical
                    # end chain.  groups run sequentially: interleaving
                    # OPEN accumulation groups inside one bank clobbers
                    # it (measured)
                    out_tt = opool.tile([128, 128], bf16)
                    pstf = psd_pool.tile([128, 512], f32, tag="ps",
                                         name="pst")
                    for i_in in range(8):
                        nc.tensor.matmul(
                            pstf[:, i_in * 16:i_in * 16 + 16],
                            wbd_t[:, 0:128],
                            tail_t[:, i_in * 16:i_in * 16 + 16],
                            start=True, stop=False)
                        nc.tensor.matmul(
                            pstf[:, i_in * 16:i_in * 16 + 16],
                            wbd_t[:, 128:256],
                            tail_t[:, (8 + i_in) * 16:(8 + i_in) * 16 + 16],
                            start=False, stop=True)
                    nc.vector.tensor_scalar_add(out_tt[:], pstf[:, 0:128],
                                                bias_t[:])
                    nc.scalar.dma_start(outt.ap(), out_tt[:])

    nc.compile()
    return nc


def _prep_consts(W, b):
    # c1-th accumulating matmul lhsT in w_bd[:, 128*c1:...]:
    # w_bd[8s+ih, 128*c1 + 16h + 2ih + c1] = W[h, s]; rest zero.
    w_bd = np.zeros((128, 256), dtype=np.float32)
    for c1 in range(2):
        for ih in range(8):
            for h in range(8):
                m = 16 * h + 2 * ih + c1
                w_bd[ih::8, 128 * c1 + m] = W[h, :]  # rows k = 8s+ih
    bias = np.repeat(np.asarray(b, np.float32), 16).reshape(128, 1)
    # pack [w_bd bf16 | bias f32] into one 516 B/partition byte blob
    cw = np.empty((128, 516), dtype=np.uint8)
    cw[:, :512] = w_bd.astype(ml_dtypes.bfloat16).view(np.uint8)
    cw[:, 512:] = np.ascontiguousarray(bias).view(np.uint8)
    return cw


def _pack(stacks, mask):
    # compacted stream: unmasked positions of the flattened [B*N*N]
    # grid in row-major order, zero-padded to CPT
    idx = np.flatnonzero(~np.asarray(mask, bool).reshape(-1))
    npos = idx.size
    assert npos <= CPT, (npos, CPT)
    st = np.asarray(stacks, np.float32).astype(IN_NP)
    st = st.transpose(1, 0, 2, 3).reshape(S, B * N * N)
    xg = np.zeros((S, CPT), dtype=IN_NP)
    xg[:, :npos] = st[:, idx]
    return xg, idx, npos


def _relayout_core(xs):
    # xs [S, CPS] -> srl [NSB,128,8192] (k=8s+ih, f=il*512+j), tail [128,256]
    m = xs[:, :NSB * SBP].reshape(S, NSB, 8, 16, 512)   # s w ih il j
    srl = np.ascontiguousarray(m.transpose(1, 0, 2, 3, 4))
    srl = srl.reshape(NSB, 128, 8192)
    t = np.ascontiguousarray(xs[:, NSB * SBP:]).reshape(S, 8, 16, 16)
    tail = t.reshape(128, 256)
    return srl, tail


def _decode_core(outd_c, outt_c):
    # outd [NSB,128,4096] p=16h+cd f=i_in*512+j -> y [H, CPS]
    y = np.empty((H, CPS), np.float32)
    d = np.asarray(outd_c).astype(np.float32)
    d = d.reshape(NSB, 8, 16, 8, 512)                   # w h cd i_in j
    y[:, :NSB * SBP] = d.transpose(1, 0, 2, 3, 4).reshape(H, NSB * SBP)
    t = np.asarray(outt_c).astype(np.float32)
    y[:, NSB * SBP:] = t.reshape(8, 16, 8, 16).reshape(H, TAILP)
    return y


def kernel(stacks, mask, W, b):
    from concourse.bass_utils import run_bass_kernel_spmd

    if "nc" not in _CACHE:
        _CACHE["nc"] = _build()
    nc = _CACHE["nc"]

    xg, idx, npos = _pack(stacks, mask)
    cw = _prep_consts(np.asarray(W, np.float32),
                      np.asarray(b, np.float32))

    in_maps = []
    for c in range(NCORES):
        srl_c, tail_c = _relayout_core(xg[:, c * CPS:(c + 1) * CPS])
        in_maps.append({"srl": srl_c, "tail": tail_c, "cw": cw})

    res = run_bass_kernel_spmd(nc, in_maps, core_ids=list(range(NCORES)),
                               **_CACHE.get("run_kwargs", {}))
    _CACHE["last_result"] = res
    y = np.concatenate(
        [_decode_core(r["outd"], r["outt"]) for r in res.results], axis=1)
    full = np.zeros((H, B * N * N), np.float32)
    full[:, idx] = y[:, :npos]
    out = np.ascontiguousarray(
        full.reshape(H, B, N, N).transpose(1, 0, 2, 3))
    return out
